# revision 6
# baseline (speedup 1.0000x reference)
"""Causal self-attention (B=4, T=2048, D=1024, H=16) on 8 TRN2 NeuronCores.

Sharding: core c -> (batch b = c//2, head-group g = c%2 of 8 heads).
Each core computes QKV projection for its 8 heads, causal attention, and a
partial out-projection (its heads' rows of W_out). The two partials per batch
are summed on the host during unshard (the "all-reduce after out_proj" of the
tensor-parallel scheme, done host-side since on-device 2-rank collectives are
slower than the host add).

Per-core program (identical SPMD on all 8 cores):
  1. Load x [2048,1024] f32, transpose on TensorE -> xT bf16 [1024(d), 2048(t)]
  2. qT/kT = (Wq|Wk)^T-stationary matmuls -> [512(feat), 2048(t)] bf16
     V     = xT-stationary matmuls -> [2048(t), 512(dv)] bf16, augmented with a
             ones column per head (softmax denominator via the same AV matmul)
  3. Per (head-pair, q-tile of 512): scoresT [k,q] in PSUM (2 heads row-packed
     in the 64x128 PE tiling), exp on ScalarE (scale=1/8) -> bf16 P^T, causal
     mask by 0/1 multiply on diag tiles, AV matmuls accumulate [65, 512]
     (64 dv rows + normalizer row), normalize via reciprocal + gpsimd
     partition-broadcast + VectorE multiply -> attnT bf16.
  4. out_proj: attnT-stationary matmuls vs W_out rows -> y partial [2048,1024].
"""

import numpy as np

import concourse.bass as bass
import concourse.mybir as mybir
import concourse.tile as tile
from concourse.bass_utils import run_bass_kernel_spmd
from concourse.masks import make_identity

F32 = mybir.dt.float32
BF16 = mybir.dt.bfloat16
AX = mybir.AluOpType

T = 2048
D = 1024
HLOC = 8          # heads per core
DKH = 64
QT = 512          # query tile
NQT = T // QT     # 4
KT = 128          # key tile
NDIN = D // 128   # 8
NMT = 4           # q/k feature m-tiles (512 local feats / 128)
VA = 65           # V cols per head incl. ones column
EXP = mybir.ActivationFunctionType.Exp


_NOP_ID = [0]


def _split_multiwaits(nc, limit=1):
    """This toolchain's walrus rejects more than one sync-wait on an
    instruction ("Too many sync wait commands"). Move excess waits onto
    same-engine NOPs inserted immediately before the instruction — the
    engine sequencer executes them in program order, so semantics are
    preserved (issue-after-wait implies execute-after-wait for DMA too)."""
    for f in nc.m.functions:
        for blk in f.blocks:
            new = []
            changed = False
            for inst in blk.instructions:
                si = inst.sync_info
                if si is not None and len(si.on_wait) > limit:
                    waits = list(si.on_wait)
                    inst.sync_info = mybir.SyncInfo(
                        on_wait=waits[:limit], on_update=list(si.on_update))
                    for w in waits[limit:]:
                        _NOP_ID[0] += 1
                        nop = mybir.InstNoOp(
                            name=f"waitnop-{_NOP_ID[0]}", ins=[], outs=[])
                        nop.engine = inst.engine
                        nop.sync_info = mybir.SyncInfo(on_wait=[w], on_update=[])
                        new.append(nop)
                    changed = True
                new.append(inst)
            if changed:
                blk.instructions = new


def build_nc():
    nc = bass.Bass()
    x_ext = nc.declare_dram_parameter("x", [T, D], F32, isOutput=False)
    w_ext = nc.declare_dram_parameter("W_qkv", [D, 3 * 512], F32, isOutput=False)
    b_ext = nc.declare_dram_parameter("b_qkv", [3 * 512], F32, isOutput=False)
    wo_ext = nc.declare_dram_parameter("W_out", [512, D], F32, isOutput=False)
    out_ext = nc.declare_dram_parameter("out", [T, D], F32, isOutput=True)

    with tile.TileContext(nc) as tc:
        with (
            tc.tile_pool(name="const", bufs=1) as constp,
            tc.tile_pool(name="big", bufs=1) as bigp,
        ):
            ident = constp.tile([128, 128], F32, tag="ident")
            make_identity(nc, ident)

            # 4 causal 0/1 masks [128, 2x512] bf16 for the diagonal k-tiles:
            # mask_i[p, (h, f)] = 1 if f >= p + 128*i else 0
            maskt = constp.tile([128, 4 * 1024], BF16, tag="maskt")
            for i in range(4):
                reg = maskt[:, i * 1024:(i + 1) * 1024]
                nc.gpsimd.memset(reg, 1.0)
                reg3 = reg.rearrange("p (h f) -> p h f", f=512)
                nc.gpsimd.affine_select(
                    out=reg3, in_=reg3,
                    compare_op=AX.is_ge, fill=0.0,
                    base=-128 * i, channel_multiplier=-1,
                    pattern=[[0, 2], [1, 512]],
                )

            # biases: per-partition vectors for q/k feature tiles, broadcast
            # tile for V (bias along the free dv axis)
            bq_sb = constp.tile([128, NMT], F32, tag="bq")
            bk_sb = constp.tile([128, NMT], F32, tag="bk")
            nc.sync.dma_start(
                out=bq_sb, in_=b_ext[0:512].rearrange("(m p) -> p m", p=128))
            nc.sync.dma_start(
                out=bk_sb, in_=b_ext[512:1024].rearrange("(m p) -> p m", p=128))
            bv_sb = constp.tile([128, 512], F32, tag="bv")
            bv_src = b_ext[1024:1536]
            nc.sync.dma_start(
                out=bv_sb,
                in_=bass.AP(tensor=bv_src.tensor, offset=bv_src.offset,
                            ap=[[0, 128]] + list(bv_src.ap)),
            )

            # persistent activations
            qT = bigp.tile([128, NMT * T], BF16, tag="qT")
            kT = bigp.tile([128, NMT * T], BF16, tag="kT")
            vaug = bigp.tile([128, (T // 128) * HLOC * VA], BF16, tag="vaug")
            attnT = bigp.tile([128, NMT * T], BF16, tag="attnT")
            woutb = bigp.tile([128, 4 * D], BF16, tag="woutb")

            # ---------------- phase A+B: transpose x, cast weights, project
            with (
                tc.tile_pool(name="proj_sb", bufs=1) as projp,
                tc.tile_pool(name="xstage", bufs=3) as xstage,
                tc.tile_pool(name="wstage", bufs=2) as wstage,
                tc.tile_pool(name="tpsum", bufs=2, space="PSUM") as tpsum,
                tc.tile_pool(name="pjpsum", bufs=4, space="PSUM") as pjpsum,
            ):
                xT = projp.tile([128, NDIN * T], BF16, tag="xT")
                wbf = projp.tile([128, NDIN * 1536], BF16, tag="wbf")

                # W_out: cast on ScalarE (idle during load phase)
                for k in range(4):
                    wot = wstage.tile([128, D], F32, tag="wo")
                    nc.sync.dma_start(out=wot, in_=wo_ext[k * 128:(k + 1) * 128, :])
                    nc.scalar.copy(woutb[:, k * D:(k + 1) * D], wot)
                # W_qkv
                for k in range(NDIN):
                    wt = wstage.tile([128, 1536], F32, tag="wqkv")
                    nc.sync.dma_start(out=wt, in_=w_ext[k * 128:(k + 1) * 128, :])
                    nc.scalar.copy(wbf[:, k * 1536:(k + 1) * 1536], wt)

                # x load + TensorE transpose (f32) + cast to bf16 on DVE
                for tt in range(T // 128):
                    xt = xstage.tile([128, D], F32, tag="x")
                    nc.sync.dma_start(out=xt, in_=x_ext[tt * 128:(tt + 1) * 128, :])
                    for g in range(2):
                        tp = tpsum.tile([128, 512], F32, tag="tp")
                        for j in range(4):
                            dj = g * 4 + j
                            nc.tensor.transpose(
                                tp[:, j * 128:(j + 1) * 128],
                                xt[:, dj * 128:(dj + 1) * 128], ident)
                        for j in range(4):
                            dj = g * 4 + j
                            nc.vector.tensor_copy(
                                xT[:, dj * T + tt * 128: dj * T + (tt + 1) * 128],
                                tp[:, j * 128:(j + 1) * 128])

                # q/k projections -> transposed feature-major layout
                for sec, dst, bias in ((0, qT, bq_sb), (1, kT, bk_sb)):
                    for mt in range(NMT):
                        for n in range(NQT):
                            ps = pjpsum.tile([128, 512], F32, tag="pj")
                            for kk in range(NDIN):
                                nc.tensor.matmul(
                                    ps,
                                    lhsT=wbf[:, kk * 1536 + sec * 512 + mt * 128:
                                             kk * 1536 + sec * 512 + (mt + 1) * 128],
                                    rhs=xT[:, kk * T + n * 512: kk * T + (n + 1) * 512],
                                    start=(kk == 0), stop=(kk == NDIN - 1),
                                )
                            nc.vector.tensor_scalar_add(
                                dst[:, mt * T + n * 512: mt * T + (n + 1) * 512],
                                ps, bias[:, mt:mt + 1])

                # V projection -> natural [t, dv], strided into vaug + ones col
                for tt in range(T // 128):
                    ps = pjpsum.tile([128, 512], F32, tag="pj")
                    for kk in range(NDIN):
                        nc.tensor.matmul(
                            ps,
                            lhsT=xT[:, kk * T + tt * 128: kk * T + (tt + 1) * 128],
                            rhs=wbf[:, kk * 1536 + 1024: kk * 1536 + 1536],
                            start=(kk == 0), stop=(kk == NDIN - 1),
                        )
                    blk = vaug[:, tt * (HLOC * VA):(tt + 1) * (HLOC * VA)]
                    blk3 = blk.rearrange("p (h c) -> p h c", c=VA)
                    nc.vector.tensor_tensor(
                        out=blk3[:, :, 0:64],
                        in0=ps.rearrange("p (h c) -> p h c", c=64),
                        in1=bv_sb.rearrange("p (h c) -> p h c", c=64),
                        op=AX.add)
                    nc.vector.memset(blk3[:, :, 64:65], 1.0)

            # ---------------- phase C: attention
            with (
                tc.tile_pool(name="scps", bufs=2, space="PSUM") as scps,
                tc.tile_pool(name="avps", bufs=4, space="PSUM") as avps,
                tc.tile_pool(name="ptp", bufs=30) as ptp,
                tc.tile_pool(name="recp", bufs=4) as recp,
                tc.tile_pool(name="dscr", bufs=4, space="DRAM") as dscr,
            ):
                units = [(hp, qt) for hp in range(HLOC // 2) for qt in range(NQT)]
                state = {}

                def emit_scores(hp, qt):
                    nkt = 4 * (qt + 1)
                    pts = []
                    for kt in range(nkt):
                        ps = scps.tile([128, 1024], F32, tag="sc")
                        for par in range(2):
                            lo, hi = par * 64, par * 64 + 64
                            nc.tensor.matmul(
                                ps[:, par * 512:(par + 1) * 512],
                                lhsT=kT[lo:hi, hp * T + kt * 128: hp * T + (kt + 1) * 128],
                                rhs=qT[lo:hi, hp * T + qt * 512: hp * T + (qt + 1) * 512],
                                start=True, stop=True,
                            )
                        pt_t = ptp.tile([128, 1024], BF16, tag="pt")
                        nc.scalar.activation(pt_t, ps, EXP, bias=0.0, scale=0.125)
                        if kt >= nkt - 4:
                            i = kt - (nkt - 4)
                            nc.vector.tensor_tensor(
                                out=pt_t, in0=pt_t,
                                in1=maskt[:, i * 1024:(i + 1) * 1024], op=AX.mult)
                        pts.append(pt_t)
                    state[(hp, qt)] = pts

                def emit_av(hp, qt):
                    nkt = 4 * (qt + 1)
                    pts = state.pop((hp, qt))
                    for par in range(2):
                        h = 2 * hp + par
                        acc = avps.tile([128, 512], F32, tag="av")
                        for kt in range(nkt):
                            nc.tensor.matmul(
                                acc[0:VA, :],
                                lhsT=vaug[:, kt * (HLOC * VA) + h * VA:
                                          kt * (HLOC * VA) + (h + 1) * VA],
                                rhs=pts[kt][:, par * 512:(par + 1) * 512],
                                start=(kt == 0), stop=(kt == nkt - 1),
                            )
                        rc = recp.tile([1, 512], F32, tag="rc")
                        rb = recp.tile([64, 512], F32, tag="rb")
                        nc.vector.reciprocal(rc, acc[64:65, :])
                        rd = dscr.tile([1, 512], F32, tag="rd")
                        nc.sync.dma_start(out=rd, in_=rc)
                        nc.sync.dma_start(
                            out=rb,
                            in_=bass.AP(tensor=rd.tensor, offset=rd.offset,
                                        ap=[[0, 64]] + list(rd.ap)[1:]))
                        nc.vector.tensor_tensor(
                            out=attnT[par * 64:(par + 1) * 64,
                                      hp * T + qt * 512: hp * T + (qt + 1) * 512],
                            in0=acc[0:64, :], in1=rb, op=AX.mult)

                for i, (hp, qt) in enumerate(units):
                    emit_scores(hp, qt)
                    if i >= 1:
                        emit_av(*units[i - 1])
                emit_av(*units[-1])

            # ---------------- phase D: out projection (partial sum over heads)
            with (
                tc.tile_pool(name="yps", bufs=4, space="PSUM") as yps,
                tc.tile_pool(name="yo", bufs=4) as yo,
            ):
                for mt in range(T // 128):
                    for n in range(2):
                        ps = yps.tile([128, 512], F32, tag="y")
                        for kk in range(4):
                            nc.tensor.matmul(
                                ps,
                                lhsT=attnT[:, kk * T + mt * 128: kk * T + (mt + 1) * 128],
                                rhs=woutb[:, kk * D + n * 512: kk * D + (n + 1) * 512],
                                start=(kk == 0), stop=(kk == 3),
                            )
                        yt = yo.tile([128, 512], F32, tag="yt")
                        nc.vector.tensor_copy(yt, ps)
                        nc.sync.dma_start(
                            out=out_ext[mt * 128:(mt + 1) * 128,
                                        n * 512:(n + 1) * 512],
                            in_=yt)
    _split_multiwaits(nc)
    return nc


_NC_CACHE = {}


def get_nc():
    if "nc" not in _NC_CACHE:
        _NC_CACHE["nc"] = build_nc()
    return _NC_CACHE["nc"]


def make_in_maps(x, W_qkv, b_qkv, W_out):
    in_maps = []
    for c in range(8):
        b, g = c // 2, c % 2
        s = slice(512 * g, 512 * (g + 1))
        wslice = np.concatenate(
            [W_qkv[:, 512 * g:512 * (g + 1)],
             W_qkv[:, 1024 + 512 * g:1024 + 512 * (g + 1)],
             W_qkv[:, 2048 + 512 * g:2048 + 512 * (g + 1)]], axis=1)
        bslice = np.concatenate(
            [b_qkv[512 * g:512 * (g + 1)],
             b_qkv[1024 + 512 * g:1024 + 512 * (g + 1)],
             b_qkv[2048 + 512 * g:2048 + 512 * (g + 1)]])
        in_maps.append({
            "x": np.ascontiguousarray(x[b], dtype=np.float32),
            "W_qkv": np.ascontiguousarray(wslice, dtype=np.float32),
            "b_qkv": np.ascontiguousarray(bslice, dtype=np.float32),
            "W_out": np.ascontiguousarray(W_out[s], dtype=np.float32),
        })
    return in_maps


def kernel(x, W_qkv, b_qkv, W_out, b_out):
    x = np.asarray(x)
    W_qkv = np.asarray(W_qkv)
    b_qkv = np.asarray(b_qkv)
    W_out = np.asarray(W_out)
    b_out = np.asarray(b_out)
    nc = get_nc()
    in_maps = make_in_maps(x, W_qkv, b_qkv, W_out)
    res = run_bass_kernel_spmd(nc, in_maps, core_ids=list(range(8))).results
    out = np.stack(
        [res[2 * b]["out"] + res[2 * b + 1]["out"] for b in range(4)], axis=0)
    out = out + b_out[None, None, :]
    return out.astype(np.float32)


# revision 10
# speedup vs baseline: 1.1406x; 1.1406x over previous
"""Causal self-attention (B=4, T=2048, D=1024, H=16) on 8 TRN2 NeuronCores.

Sharding: core c -> (batch b = c//2, head-group g = c%2 of 8 heads).
Each core computes QKV projection for its 8 heads, causal attention, and a
partial out-projection (its heads' rows of W_out). The two partials per batch
are summed on the host during unshard (the "all-reduce after out_proj" of the
tensor-parallel scheme, done host-side since on-device 2-rank collectives are
slower than the host add).

Per-core program (identical SPMD on all 8 cores):
  1. Load x [2048,1024] f32, transpose on TensorE -> xT bf16 [1024(d), 2048(t)]
  2. qT/kT = (Wq|Wk)^T-stationary matmuls -> [512(feat), 2048(t)] bf16
     V     = xT-stationary matmuls -> [2048(t), 512(dv)] bf16, augmented with a
             ones column per head (softmax denominator via the same AV matmul)
  3. Per (head-pair, q-tile of 512): scoresT [k,q] in PSUM (2 heads row-packed
     in the 64x128 PE tiling), exp on ScalarE (scale=1/8) -> bf16 P^T, causal
     mask by 0/1 multiply on diag tiles, AV matmuls accumulate [65, 512]
     (64 dv rows + normalizer row), normalize via reciprocal + gpsimd
     partition-broadcast + VectorE multiply -> attnT bf16.
  4. out_proj: attnT-stationary matmuls vs W_out rows -> y partial [2048,1024].
"""

import numpy as np

import concourse.bass as bass
import concourse.mybir as mybir
import concourse.tile as tile
from concourse.bass_utils import run_bass_kernel_spmd
from concourse.masks import make_identity

F32 = mybir.dt.float32
BF16 = mybir.dt.bfloat16
AX = mybir.AluOpType

T = 2048
D = 1024
HLOC = 8          # heads per core
DKH = 64
QT = 512          # query tile
NQT = T // QT     # 4
KT = 128          # key tile
NDIN = D // 128   # 8
NMT = 4           # q/k feature m-tiles (512 local feats / 128)
VA = 65           # V cols per head incl. ones column
EXP = mybir.ActivationFunctionType.Exp


_NOP_ID = [0]


def _split_multiwaits(nc, limit=1):
    """This toolchain's walrus rejects more than one sync-wait on an
    instruction ("Too many sync wait commands"). Move excess waits onto
    same-engine NOPs inserted immediately before the instruction — the
    engine sequencer executes them in program order, so semantics are
    preserved (issue-after-wait implies execute-after-wait for DMA too)."""
    for f in nc.m.functions:
        for blk in f.blocks:
            new = []
            changed = False
            for inst in blk.instructions:
                si = inst.sync_info
                if si is not None and len(si.on_wait) > limit:
                    waits = list(si.on_wait)
                    inst.sync_info = mybir.SyncInfo(
                        on_wait=waits[:limit], on_update=list(si.on_update))
                    for w in waits[limit:]:
                        _NOP_ID[0] += 1
                        nop = mybir.InstNoOp(
                            name=f"waitnop-{_NOP_ID[0]}", ins=[], outs=[])
                        nop.engine = inst.engine
                        nop.sync_info = mybir.SyncInfo(on_wait=[w], on_update=[])
                        new.append(nop)
                    changed = True
                new.append(inst)
            if changed:
                blk.instructions = new


def build_nc():
    nc = bass.Bass()
    x_ext = nc.declare_dram_parameter("x", [T, D], F32, isOutput=False)
    w_ext = nc.declare_dram_parameter("W_qkv", [D, 3 * 512], F32, isOutput=False)
    b_ext = nc.declare_dram_parameter("b_qkv", [3 * 512], F32, isOutput=False)
    wo_ext = nc.declare_dram_parameter("W_out", [512, D], F32, isOutput=False)
    out_ext = nc.declare_dram_parameter("out", [T, D], F32, isOutput=True)

    with tile.TileContext(nc) as tc:
        with (
            tc.tile_pool(name="const", bufs=1) as constp,
            tc.tile_pool(name="big", bufs=1) as bigp,
        ):
            ident = constp.tile([128, 128], F32, tag="ident")
            make_identity(nc, ident)

            # causal 0/1 mask [128, 2x512] bf16 (head-duplicated triangle):
            # mask[p, (h, f)] = 1 if f >= p else 0 — diagonal k-tiles reduce to
            # this one pattern after live-range slicing.
            maskt = constp.tile([128, 1024], BF16, tag="maskt")
            nc.gpsimd.memset(maskt, 1.0)
            mk3 = maskt.rearrange("p (h f) -> p h f", f=512)
            nc.gpsimd.affine_select(
                out=mk3, in_=mk3,
                compare_op=AX.is_ge, fill=0.0,
                base=0, channel_multiplier=-1,
                pattern=[[0, 2], [1, 512]],
            )

            # biases: per-partition vectors for q/k feature tiles, broadcast
            # tile for V (bias along the free dv axis)
            bq_sb = constp.tile([128, NMT], F32, tag="bq")
            bk_sb = constp.tile([128, NMT], F32, tag="bk")
            nc.sync.dma_start(
                out=bq_sb, in_=b_ext[0:512].rearrange("(m p) -> p m", p=128))
            nc.sync.dma_start(
                out=bk_sb, in_=b_ext[512:1024].rearrange("(m p) -> p m", p=128))
            bv_sb = constp.tile([128, 512], F32, tag="bv")
            bv_src = b_ext[1024:1536]
            nc.sync.dma_start(
                out=bv_sb,
                in_=bass.AP(tensor=bv_src.tensor, offset=bv_src.offset,
                            ap=[[0, 128]] + list(bv_src.ap)),
            )

            # persistent activations
            qT = bigp.tile([128, NMT * T], BF16, tag="qT")
            kT = bigp.tile([128, NMT * T], BF16, tag="kT")
            vaug = bigp.tile([128, (T // 128) * HLOC * VA], BF16, tag="vaug")
            attnT = bigp.tile([128, NMT * T], BF16, tag="attnT")
            woutb = bigp.tile([128, 4 * D], BF16, tag="woutb")

            # ------- phases B+C interleaved: transpose x, project, attention
            with (
                tc.tile_pool(name="proj_sb", bufs=1) as projp,
                tc.tile_pool(name="pjpsum", bufs=2, space="PSUM") as pjpsum,
            ):
                xT = projp.tile([128, NDIN * T], BF16, tag="xT")
                wbf = projp.tile([128, NDIN * 1536], BF16, tag="wbf")

                # x load + TensorE transpose (f32) + cast to bf16 on DVE;
                # staging pools close before the attention pools open
                with (
                    tc.tile_pool(name="xstage", bufs=3) as xstage,
                    tc.tile_pool(name="wstage", bufs=2) as wstage,
                    tc.tile_pool(name="tpsum", bufs=2, space="PSUM") as tpsum,
                ):
                    for tt in range(T // 128):
                        xt = xstage.tile([128, D], F32, tag="x")
                        nc.sync.dma_start(
                            out=xt, in_=x_ext[tt * 128:(tt + 1) * 128, :])
                        for g in range(2):
                            tp = tpsum.tile([128, 512], F32, tag="tp")
                            for j in range(4):
                                dj = g * 4 + j
                                nc.tensor.transpose(
                                    tp[:, j * 128:(j + 1) * 128],
                                    xt[:, dj * 128:(dj + 1) * 128], ident)
                            for j in range(4):
                                dj = g * 4 + j
                                nc.vector.tensor_copy(
                                    xT[:, dj * T + tt * 128: dj * T + (tt + 1) * 128],
                                    tp[:, j * 128:(j + 1) * 128])

                    # weight loads + casts (ScalarE is idle this early)
                    for k in range(4):
                        wot = wstage.tile([128, D], F32, tag="wo")
                        nc.sync.dma_start(out=wot, in_=wo_ext[k * 128:(k + 1) * 128, :])
                        nc.scalar.copy(woutb[:, k * D:(k + 1) * D], wot)
                    for k in range(NDIN):
                        wt = wstage.tile([128, 1536], F32, tag="wqkv")
                        nc.sync.dma_start(out=wt, in_=w_ext[k * 128:(k + 1) * 128, :])
                        nc.scalar.copy(wbf[:, k * 1536:(k + 1) * 1536], wt)

                def emit_qkproj(mt):
                    for sec, dst, bias in ((0, qT, bq_sb), (1, kT, bk_sb)):
                        for n in range(NQT):
                            ps = pjpsum.tile([128, 512], F32, tag="pj")
                            for kk in range(NDIN):
                                nc.tensor.matmul(
                                    ps,
                                    lhsT=wbf[:, kk * 1536 + sec * 512 + mt * 128:
                                             kk * 1536 + sec * 512 + (mt + 1) * 128],
                                    rhs=xT[:, kk * T + n * 512: kk * T + (n + 1) * 512],
                                    start=(kk == 0), stop=(kk == NDIN - 1),
                                )
                            nc.vector.tensor_scalar_add(
                                dst[:, mt * T + n * 512: mt * T + (n + 1) * 512],
                                ps, bias[:, mt:mt + 1])

                def emit_vproj():
                    for tt in range(T // 128):
                        ps = pjpsum.tile([128, 512], F32, tag="pj")
                        for kk in range(NDIN):
                            nc.tensor.matmul(
                                ps,
                                lhsT=xT[:, kk * T + tt * 128: kk * T + (tt + 1) * 128],
                                rhs=wbf[:, kk * 1536 + 1024: kk * 1536 + 1536],
                                start=(kk == 0), stop=(kk == NDIN - 1),
                            )
                        blk = vaug[:, tt * (HLOC * VA):(tt + 1) * (HLOC * VA)]
                        blk3 = blk.rearrange("p (h c) -> p h c", c=VA)
                        nc.vector.tensor_tensor(
                            out=blk3[:, :, 0:64],
                            in0=ps.rearrange("p (h c) -> p h c", c=64),
                            in1=bv_sb.rearrange("p (h c) -> p h c", c=64),
                            op=AX.add)
                        nc.vector.memset(blk3[:, :, 64:65], 1.0)

                with (
                    tc.tile_pool(name="scps", bufs=2, space="PSUM") as scps,
                    tc.tile_pool(name="avps", bufs=2, space="PSUM") as avps,
                    tc.tile_pool(name="ptp", bufs=22) as ptp,
                    tc.tile_pool(name="accp", bufs=3) as accp,
                    tc.tile_pool(name="recp", bufs=2) as recp,
                    tc.tile_pool(name="dscr", bufs=4, space="DRAM") as dscr,
                ):
                    def emit_unit(hp, qt):
                        nkt = 4 * (qt + 1)
                        accs = [avps.tile([128, 512], F32, tag="av",
                                          name=f"av{par}")
                                for par in range(2)]
                        m3 = maskt.rearrange("p (h q) -> p h q", q=512)
                        for c0 in range(0, nkt, 8):
                            chunk = list(range(c0, min(c0 + 8, nkt)))
                            pts = {}
                            for kt in chunk:
                                # diagonal k-tile i (k0 = q0+128i): only queries
                                # f >= 128i are live; compute/exp/mask that range
                                i = kt - (nkt - 4)
                                lo_q = max(0, 128 * i)
                                ps = scps.tile([128, 1024], F32, tag="sc")
                                for par in range(2):
                                    lo, hi = par * 64, par * 64 + 64
                                    nc.tensor.matmul(
                                        ps[:, par * 512 + lo_q:(par + 1) * 512],
                                        lhsT=kT[lo:hi, hp * T + kt * 128:
                                                hp * T + (kt + 1) * 128],
                                        rhs=qT[lo:hi, hp * T + qt * 512 + lo_q:
                                               hp * T + (qt + 1) * 512],
                                        start=True, stop=True,
                                    )
                                pt_t = ptp.tile([128, 1024], BF16, tag="pt")
                                ps3 = ps.rearrange("p (h q) -> p h q", q=512)
                                pt3 = pt_t.rearrange("p (h q) -> p h q", q=512)
                                nc.scalar.activation(
                                    pt3[:, :, lo_q:512], ps3[:, :, lo_q:512],
                                    EXP, bias=0.0, scale=0.125)
                                if i >= 0:
                                    nc.vector.tensor_tensor(
                                        out=pt3[:, :, lo_q:512],
                                        in0=pt3[:, :, lo_q:512],
                                        in1=m3[:, :, 0:512 - lo_q], op=AX.mult)
                                pts[kt] = (pt_t, lo_q)
                            for par in range(2):
                                h = 2 * hp + par
                                for kt in chunk:
                                    pt_t, lo_q = pts[kt]
                                    nc.tensor.matmul(
                                        accs[par][0:VA, lo_q:512],
                                        lhsT=vaug[:, kt * (HLOC * VA) + h * VA:
                                                  kt * (HLOC * VA) + (h + 1) * VA],
                                        rhs=pt_t[:, par * 512 + lo_q:(par + 1) * 512],
                                        start=(kt == 0), stop=(kt == nkt - 1),
                                    )
                        for par in range(2):
                            acc = accs[par]
                            accsb = accp.tile([VA, 512], F32, tag="accs")
                            nc.vector.tensor_copy(accsb, acc[0:VA, :])
                            rc = recp.tile([1, 512], F32, tag="rc")
                            rb = recp.tile([64, 512], F32, tag="rb")
                            nc.vector.reciprocal(rc, accsb[64:65, :])
                            rd = dscr.tile([1, 512], F32, tag="rd")
                            nc.sync.dma_start(out=rd, in_=rc)
                            nc.sync.dma_start(
                                out=rb,
                                in_=bass.AP(tensor=rd.tensor, offset=rd.offset,
                                            ap=[[0, 64]] + list(rd.ap)[1:]))
                            nc.vector.tensor_tensor(
                                out=attnT[par * 64:(par + 1) * 64,
                                          hp * T + qt * 512: hp * T + (qt + 1) * 512],
                                in0=accsb[0:64, :], in1=rb, op=AX.mult)

                    emit_qkproj(0)
                    emit_vproj()
                    for hp in range(HLOC // 2):
                        if hp >= 1:
                            emit_qkproj(hp)
                        for qt in range(NQT):
                            emit_unit(hp, qt)

            # ---------------- phase D: out projection (partial sum over heads)
            with (
                tc.tile_pool(name="yps", bufs=4, space="PSUM") as yps,
                tc.tile_pool(name="yo", bufs=4) as yo,
            ):
                for mt in range(T // 128):
                    for n in range(2):
                        ps = yps.tile([128, 512], F32, tag="y")
                        for kk in range(4):
                            nc.tensor.matmul(
                                ps,
                                lhsT=attnT[:, kk * T + mt * 128: kk * T + (mt + 1) * 128],
                                rhs=woutb[:, kk * D + n * 512: kk * D + (n + 1) * 512],
                                start=(kk == 0), stop=(kk == 3),
                            )
                        yt = yo.tile([128, 512], F32, tag="yt")
                        nc.vector.tensor_copy(yt, ps)
                        nc.sync.dma_start(
                            out=out_ext[mt * 128:(mt + 1) * 128,
                                        n * 512:(n + 1) * 512],
                            in_=yt)
    _split_multiwaits(nc)
    return nc


_NC_CACHE = {}


def get_nc():
    if "nc" not in _NC_CACHE:
        _NC_CACHE["nc"] = build_nc()
    return _NC_CACHE["nc"]


def make_in_maps(x, W_qkv, b_qkv, W_out):
    in_maps = []
    for c in range(8):
        b, g = c // 2, c % 2
        s = slice(512 * g, 512 * (g + 1))
        wslice = np.concatenate(
            [W_qkv[:, 512 * g:512 * (g + 1)],
             W_qkv[:, 1024 + 512 * g:1024 + 512 * (g + 1)],
             W_qkv[:, 2048 + 512 * g:2048 + 512 * (g + 1)]], axis=1)
        bslice = np.concatenate(
            [b_qkv[512 * g:512 * (g + 1)],
             b_qkv[1024 + 512 * g:1024 + 512 * (g + 1)],
             b_qkv[2048 + 512 * g:2048 + 512 * (g + 1)]])
        in_maps.append({
            "x": np.ascontiguousarray(x[b], dtype=np.float32),
            "W_qkv": np.ascontiguousarray(wslice, dtype=np.float32),
            "b_qkv": np.ascontiguousarray(bslice, dtype=np.float32),
            "W_out": np.ascontiguousarray(W_out[s], dtype=np.float32),
        })
    return in_maps


def kernel(x, W_qkv, b_qkv, W_out, b_out):
    x = np.asarray(x)
    W_qkv = np.asarray(W_qkv)
    b_qkv = np.asarray(b_qkv)
    W_out = np.asarray(W_out)
    b_out = np.asarray(b_out)
    nc = get_nc()
    in_maps = make_in_maps(x, W_qkv, b_qkv, W_out)
    res = run_bass_kernel_spmd(nc, in_maps, core_ids=list(range(8))).results
    out = np.stack(
        [res[2 * b]["out"] + res[2 * b + 1]["out"] for b in range(4)], axis=0)
    out = out + b_out[None, None, :]
    return out.astype(np.float32)


# revision 11
# speedup vs baseline: 1.1502x; 1.0084x over previous
"""Causal self-attention (B=4, T=2048, D=1024, H=16) on 8 TRN2 NeuronCores.

Sharding: core c -> (batch b = c//2, head-group g = c%2 of 8 heads).
Each core computes QKV projection for its 8 heads, causal attention, and a
partial out-projection (its heads' rows of W_out). The two partials per batch
are summed on the host during unshard (the "all-reduce after out_proj" of the
tensor-parallel scheme, done host-side since on-device 2-rank collectives are
slower than the host add).

Per-core program (identical SPMD on all 8 cores):
  1. Load x [2048,1024] f32, transpose on TensorE -> xT bf16 [1024(d), 2048(t)]
  2. qT/kT = (Wq|Wk)^T-stationary matmuls -> [512(feat), 2048(t)] bf16
     V     = xT-stationary matmuls -> [2048(t), 512(dv)] bf16, augmented with a
             ones column per head (softmax denominator via the same AV matmul)
  3. Per (head-pair, q-tile of 512): scoresT [k,q] in PSUM (2 heads row-packed
     in the 64x128 PE tiling), exp on ScalarE (scale=1/8) -> bf16 P^T, causal
     mask by 0/1 multiply on diag tiles, AV matmuls accumulate [65, 512]
     (64 dv rows + normalizer row), normalize via reciprocal + gpsimd
     partition-broadcast + VectorE multiply -> attnT bf16.
  4. out_proj: attnT-stationary matmuls vs W_out rows -> y partial [2048,1024].
"""

import numpy as np

import concourse.bass as bass
import concourse.mybir as mybir
import concourse.tile as tile
from concourse.bass_utils import run_bass_kernel_spmd
from concourse.masks import make_identity

F32 = mybir.dt.float32
BF16 = mybir.dt.bfloat16
AX = mybir.AluOpType

T = 2048
D = 1024
HLOC = 8          # heads per core
DKH = 64
QT = 512          # query tile
NQT = T // QT     # 4
KT = 128          # key tile
NDIN = D // 128   # 8
NMT = 4           # q/k feature m-tiles (512 local feats / 128)
VA = 65           # V cols per head incl. ones column
EXP = mybir.ActivationFunctionType.Exp


_NOP_ID = [0]


def _split_multiwaits(nc, limit=1):
    """This toolchain's walrus rejects more than one sync-wait on an
    instruction ("Too many sync wait commands"). Move excess waits onto
    same-engine NOPs inserted immediately before the instruction — the
    engine sequencer executes them in program order, so semantics are
    preserved (issue-after-wait implies execute-after-wait for DMA too)."""
    for f in nc.m.functions:
        for blk in f.blocks:
            new = []
            changed = False
            for inst in blk.instructions:
                si = inst.sync_info
                if si is not None and len(si.on_wait) > limit:
                    waits = list(si.on_wait)
                    inst.sync_info = mybir.SyncInfo(
                        on_wait=waits[:limit], on_update=list(si.on_update))
                    for w in waits[limit:]:
                        _NOP_ID[0] += 1
                        nop = mybir.InstNoOp(
                            name=f"waitnop-{_NOP_ID[0]}", ins=[], outs=[])
                        nop.engine = inst.engine
                        nop.sync_info = mybir.SyncInfo(on_wait=[w], on_update=[])
                        new.append(nop)
                    changed = True
                new.append(inst)
            if changed:
                blk.instructions = new


def build_nc():
    nc = bass.Bass()
    x_ext = nc.declare_dram_parameter("x", [T, D], F32, isOutput=False)
    w_ext = nc.declare_dram_parameter("W_qkv", [D, 3 * 512], F32, isOutput=False)
    b_ext = nc.declare_dram_parameter("b_qkv", [3 * 512], F32, isOutput=False)
    wo_ext = nc.declare_dram_parameter("W_out", [512, D], F32, isOutput=False)
    out_ext = nc.declare_dram_parameter("out", [T, D], F32, isOutput=True)

    with tile.TileContext(nc) as tc:
        with (
            tc.tile_pool(name="const", bufs=1) as constp,
            tc.tile_pool(name="big", bufs=1) as bigp,
        ):
            ident = constp.tile([128, 128], F32, tag="ident")
            make_identity(nc, ident)

            # causal 0/1 mask [128, 2x512] bf16 (head-duplicated triangle):
            # mask[p, (h, f)] = 1 if f >= p else 0 — diagonal k-tiles reduce to
            # this one pattern after live-range slicing.
            maskt = constp.tile([128, 1024], BF16, tag="maskt")
            nc.gpsimd.memset(maskt, 1.0)
            mk3 = maskt.rearrange("p (h f) -> p h f", f=512)
            nc.gpsimd.affine_select(
                out=mk3, in_=mk3,
                compare_op=AX.is_ge, fill=0.0,
                base=0, channel_multiplier=-1,
                pattern=[[0, 2], [1, 512]],
            )

            # biases: per-partition vectors for q/k feature tiles, broadcast
            # tile for V (bias along the free dv axis)
            bq_sb = constp.tile([128, NMT], F32, tag="bq")
            bk_sb = constp.tile([128, NMT], F32, tag="bk")
            nc.sync.dma_start(
                out=bq_sb, in_=b_ext[0:512].rearrange("(m p) -> p m", p=128))
            nc.sync.dma_start(
                out=bk_sb, in_=b_ext[512:1024].rearrange("(m p) -> p m", p=128))
            bv_sb = constp.tile([128, 512], F32, tag="bv")
            bv_src = b_ext[1024:1536]
            nc.sync.dma_start(
                out=bv_sb,
                in_=bass.AP(tensor=bv_src.tensor, offset=bv_src.offset,
                            ap=[[0, 128]] + list(bv_src.ap)),
            )

            # persistent activations
            qT = bigp.tile([128, NMT * T], BF16, tag="qT")
            kT = bigp.tile([128, NMT * T], BF16, tag="kT")
            vaug = bigp.tile([128, (T // 128) * HLOC * VA], BF16, tag="vaug")
            attnT = bigp.tile([128, NMT * T], BF16, tag="attnT")
            woutb = bigp.tile([128, 4 * D], BF16, tag="woutb")

            # ------- phases B+C interleaved: transpose x, project, attention
            with (
                tc.tile_pool(name="proj_sb", bufs=1) as projp,
                tc.tile_pool(name="pjpsum", bufs=2, space="PSUM") as pjpsum,
            ):
                xT = projp.tile([128, NDIN * T], BF16, tag="xT")
                wbf = projp.tile([128, NDIN * 1536], BF16, tag="wbf")

                # x load + TensorE transpose (f32) + cast to bf16 on DVE;
                # staging pools close before the attention pools open
                with (
                    tc.tile_pool(name="xstage", bufs=3) as xstage,
                    tc.tile_pool(name="wstage", bufs=2) as wstage,
                    tc.tile_pool(name="tpsum", bufs=2, space="PSUM") as tpsum,
                ):
                    for tt in range(T // 128):
                        xt = xstage.tile([128, D], F32, tag="x")
                        nc.sync.dma_start(
                            out=xt, in_=x_ext[tt * 128:(tt + 1) * 128, :])
                        for g in range(2):
                            tp = tpsum.tile([128, 512], F32, tag="tp")
                            for j in range(4):
                                dj = g * 4 + j
                                nc.tensor.transpose(
                                    tp[:, j * 128:(j + 1) * 128],
                                    xt[:, dj * 128:(dj + 1) * 128], ident)
                            xT_dst = bass.AP(
                                tensor=xT.tensor,
                                offset=xT.offset + (g * 4) * T + tt * 128,
                                ap=[list(xT.ap[0]), [T, 4], [1, 128]])
                            nc.vector.tensor_copy(
                                xT_dst,
                                tp.rearrange("p (j t) -> p j t", t=128))

                    # weight loads + casts (ScalarE is idle this early)
                    for k in range(4):
                        wot = wstage.tile([128, D], F32, tag="wo")
                        nc.sync.dma_start(out=wot, in_=wo_ext[k * 128:(k + 1) * 128, :])
                        nc.scalar.copy(woutb[:, k * D:(k + 1) * D], wot)
                    for k in range(NDIN):
                        wt = wstage.tile([128, 1536], F32, tag="wqkv")
                        nc.sync.dma_start(out=wt, in_=w_ext[k * 128:(k + 1) * 128, :])
                        nc.scalar.copy(wbf[:, k * 1536:(k + 1) * 1536], wt)

                def emit_qkproj(mt):
                    for sec, dst, bias in ((0, qT, bq_sb), (1, kT, bk_sb)):
                        for n in range(NQT):
                            ps = pjpsum.tile([128, 512], F32, tag="pj")
                            for kk in range(NDIN):
                                nc.tensor.matmul(
                                    ps,
                                    lhsT=wbf[:, kk * 1536 + sec * 512 + mt * 128:
                                             kk * 1536 + sec * 512 + (mt + 1) * 128],
                                    rhs=xT[:, kk * T + n * 512: kk * T + (n + 1) * 512],
                                    start=(kk == 0), stop=(kk == NDIN - 1),
                                )
                            nc.vector.tensor_scalar_add(
                                dst[:, mt * T + n * 512: mt * T + (n + 1) * 512],
                                ps, bias[:, mt:mt + 1])

                def emit_vproj(tts):
                    for tt in tts:
                        ps = pjpsum.tile([128, 512], F32, tag="pj")
                        for kk in range(NDIN):
                            nc.tensor.matmul(
                                ps,
                                lhsT=xT[:, kk * T + tt * 128: kk * T + (tt + 1) * 128],
                                rhs=wbf[:, kk * 1536 + 1024: kk * 1536 + 1536],
                                start=(kk == 0), stop=(kk == NDIN - 1),
                            )
                        blk = vaug[:, tt * (HLOC * VA):(tt + 1) * (HLOC * VA)]
                        blk3 = blk.rearrange("p (h c) -> p h c", c=VA)
                        nc.vector.tensor_tensor(
                            out=blk3[:, :, 0:64],
                            in0=ps.rearrange("p (h c) -> p h c", c=64),
                            in1=bv_sb.rearrange("p (h c) -> p h c", c=64),
                            op=AX.add)
                        nc.vector.memset(blk3[:, :, 64:65], 1.0)

                with (
                    tc.tile_pool(name="scps", bufs=2, space="PSUM") as scps,
                    tc.tile_pool(name="avps", bufs=2, space="PSUM") as avps,
                    tc.tile_pool(name="ptp", bufs=22) as ptp,
                    tc.tile_pool(name="accp", bufs=3) as accp,
                    tc.tile_pool(name="recp", bufs=2) as recp,
                    tc.tile_pool(name="dscr", bufs=4, space="DRAM") as dscr,
                ):
                    def emit_unit(hp, qt):
                        nkt = 4 * (qt + 1)
                        accs = [avps.tile([128, 512], F32, tag="av",
                                          name=f"av{par}")
                                for par in range(2)]
                        m3 = maskt.rearrange("p (h q) -> p h q", q=512)
                        for c0 in range(0, nkt, 8):
                            chunk = list(range(c0, min(c0 + 8, nkt)))
                            pts = {}
                            for kt in chunk:
                                # diagonal k-tile i (k0 = q0+128i): only queries
                                # f >= 128i are live; compute/exp/mask that range
                                i = kt - (nkt - 4)
                                lo_q = max(0, 128 * i)
                                ps = scps.tile([128, 1024], F32, tag="sc")
                                for par in range(2):
                                    lo, hi = par * 64, par * 64 + 64
                                    nc.tensor.matmul(
                                        ps[:, par * 512 + lo_q:(par + 1) * 512],
                                        lhsT=kT[lo:hi, hp * T + kt * 128:
                                                hp * T + (kt + 1) * 128],
                                        rhs=qT[lo:hi, hp * T + qt * 512 + lo_q:
                                               hp * T + (qt + 1) * 512],
                                        start=True, stop=True,
                                    )
                                pt_t = ptp.tile([128, 1024], BF16, tag="pt")
                                ps3 = ps.rearrange("p (h q) -> p h q", q=512)
                                pt3 = pt_t.rearrange("p (h q) -> p h q", q=512)
                                nc.scalar.activation(
                                    pt3[:, :, lo_q:512], ps3[:, :, lo_q:512],
                                    EXP, bias=0.0, scale=0.125)
                                if i >= 0:
                                    nc.vector.tensor_tensor(
                                        out=pt3[:, :, lo_q:512],
                                        in0=pt3[:, :, lo_q:512],
                                        in1=m3[:, :, 0:512 - lo_q], op=AX.mult)
                                pts[kt] = (pt_t, lo_q)
                            for par in range(2):
                                h = 2 * hp + par
                                for kt in chunk:
                                    pt_t, lo_q = pts[kt]
                                    nc.tensor.matmul(
                                        accs[par][0:VA, lo_q:512],
                                        lhsT=vaug[:, kt * (HLOC * VA) + h * VA:
                                                  kt * (HLOC * VA) + (h + 1) * VA],
                                        rhs=pt_t[:, par * 512 + lo_q:(par + 1) * 512],
                                        start=(kt == 0), stop=(kt == nkt - 1),
                                    )
                        for par in range(2):
                            acc = accs[par]
                            accsb = accp.tile([VA, 512], F32, tag="accs")
                            nc.vector.tensor_copy(accsb, acc[0:VA, :])
                            rc = recp.tile([1, 512], F32, tag="rc")
                            rb = recp.tile([64, 512], F32, tag="rb")
                            nc.vector.reciprocal(rc, accsb[64:65, :])
                            rd = dscr.tile([1, 512], F32, tag="rd")
                            nc.sync.dma_start(out=rd, in_=rc)
                            nc.sync.dma_start(
                                out=rb,
                                in_=bass.AP(tensor=rd.tensor, offset=rd.offset,
                                            ap=[[0, 64]] + list(rd.ap)[1:]))
                            nc.vector.tensor_tensor(
                                out=attnT[par * 64:(par + 1) * 64,
                                          hp * T + qt * 512: hp * T + (qt + 1) * 512],
                                in0=accsb[0:64, :], in1=rb, op=AX.mult)

                    emit_qkproj(0)
                    for hp in range(HLOC // 2):
                        if hp >= 1:
                            emit_qkproj(hp)
                        for qt in range(NQT):
                            if hp == 0:
                                emit_vproj(range(4 * qt, 4 * qt + 4))
                            emit_unit(hp, qt)

            # ---------------- phase D: out projection (partial sum over heads)
            with (
                tc.tile_pool(name="yps", bufs=2, space="PSUM") as yps,
                tc.tile_pool(name="yo", bufs=3) as yo,
            ):
                for mt in range(T // 128):
                    ps = yps.tile([128, 1024], F32, tag="y")
                    for n in range(2):
                        for kk in range(4):
                            nc.tensor.matmul(
                                ps[:, n * 512:(n + 1) * 512],
                                lhsT=attnT[:, kk * T + mt * 128: kk * T + (mt + 1) * 128],
                                rhs=woutb[:, kk * D + n * 512: kk * D + (n + 1) * 512],
                                start=(kk == 0), stop=(kk == 3),
                            )
                    yt = yo.tile([128, 1024], F32, tag="yt")
                    nc.vector.tensor_copy(yt, ps)
                    nc.sync.dma_start(
                        out=out_ext[mt * 128:(mt + 1) * 128, :], in_=yt)
    _split_multiwaits(nc)
    return nc


_NC_CACHE = {}


def get_nc():
    if "nc" not in _NC_CACHE:
        _NC_CACHE["nc"] = build_nc()
    return _NC_CACHE["nc"]


def make_in_maps(x, W_qkv, b_qkv, W_out):
    in_maps = []
    for c in range(8):
        b, g = c // 2, c % 2
        s = slice(512 * g, 512 * (g + 1))
        wslice = np.concatenate(
            [W_qkv[:, 512 * g:512 * (g + 1)],
             W_qkv[:, 1024 + 512 * g:1024 + 512 * (g + 1)],
             W_qkv[:, 2048 + 512 * g:2048 + 512 * (g + 1)]], axis=1)
        bslice = np.concatenate(
            [b_qkv[512 * g:512 * (g + 1)],
             b_qkv[1024 + 512 * g:1024 + 512 * (g + 1)],
             b_qkv[2048 + 512 * g:2048 + 512 * (g + 1)]])
        in_maps.append({
            "x": np.ascontiguousarray(x[b], dtype=np.float32),
            "W_qkv": np.ascontiguousarray(wslice, dtype=np.float32),
            "b_qkv": np.ascontiguousarray(bslice, dtype=np.float32),
            "W_out": np.ascontiguousarray(W_out[s], dtype=np.float32),
        })
    return in_maps


def kernel(x, W_qkv, b_qkv, W_out, b_out):
    x = np.asarray(x)
    W_qkv = np.asarray(W_qkv)
    b_qkv = np.asarray(b_qkv)
    W_out = np.asarray(W_out)
    b_out = np.asarray(b_out)
    nc = get_nc()
    in_maps = make_in_maps(x, W_qkv, b_qkv, W_out)
    res = run_bass_kernel_spmd(nc, in_maps, core_ids=list(range(8))).results
    out = np.stack(
        [res[2 * b]["out"] + res[2 * b + 1]["out"] for b in range(4)], axis=0)
    out = out + b_out[None, None, :]
    return out.astype(np.float32)


# revision 36
# speedup vs baseline: 1.1865x; 1.0315x over previous
"""Causal self-attention (B=4, T=2048, D=1024, H=16) on 8 TRN2 NeuronCores.

Sharding: core c -> (batch b = c//2, head-group g = c%2 of 8 heads).
Each core computes QKV projection for its 8 heads, causal attention, and a
partial out-projection (its heads' rows of W_out). The two partials per batch
are summed on the host during unshard (the "all-reduce after out_proj" of the
tensor-parallel scheme, done host-side since on-device 2-rank collectives are
slower than the host add).

Per-core program (identical SPMD on all 8 cores):
  1. Load x [2048,1024] f32, transpose on TensorE -> xT bf16 [1024(d), 2048(t)]
  2. qT/kT = (Wq|Wk)^T-stationary matmuls -> [512(feat), 2048(t)] bf16
     V     = xT-stationary matmuls -> [2048(t), 512(dv)] bf16, augmented with a
             ones column per head (softmax denominator via the same AV matmul)
  3. Per (head-pair, q-tile of 512): scoresT [k,q] in PSUM (2 heads row-packed
     into the 64x128 PE tiling via partition-half placement), exp on ScalarE
     (scale=1/8, fp32 in -> bf16 out), causal 0/1-mask multiply on diagonal
     tiles (live query sub-ranges only), AV matmuls accumulate [65, 512]
     (64 dv rows + the softmax-denominator row from the ones column),
     normalize via VectorE reciprocal + DRAM-bounce broadcast DMA + VectorE
     multiply -> attnT bf16 [dv, q].
  4. out_proj: attnT-stationary matmuls vs W_out rows -> y partial, DMA out.
     Emitted one q-tile behind the last head-pair's attention so it fills
     TensorE stalls and hides the tail.

Projections for head-pair j+1 are emitted between attention units so the
TensorE fills exp-latency stalls with projection matmuls. Peak engine usage
(cost-model): TensorE ~242us busy of ~297us total; ScalarE ~172us; VectorE
~150us; DMA ~83us.
"""

import numpy as np

import concourse.bass as bass
import concourse.mybir as mybir
import concourse.tile as tile
from concourse.bass_utils import run_bass_kernel_spmd
from concourse.masks import make_identity

F32 = mybir.dt.float32
BF16 = mybir.dt.bfloat16
AX = mybir.AluOpType

T = 2048
D = 1024
HLOC = 8          # heads per core
DKH = 64
QT = 512          # query tile
NQT = T // QT     # 4
KT = 128          # key tile
NDIN = D // 128   # 8
NMT = 4           # q/k feature m-tiles (512 local feats / 128)
VA = 65           # V cols per head incl. ones column
EXP = mybir.ActivationFunctionType.Exp


_NOP_ID = [0]


def _split_multiwaits(nc, limit=1):
    """This toolchain's walrus rejects more than one sync-wait on an
    instruction ("Too many sync wait commands"). Move excess waits onto
    same-engine NOPs inserted immediately before the instruction — the
    engine sequencer executes them in program order, so semantics are
    preserved (issue-after-wait implies execute-after-wait for DMA too)."""
    for f in nc.m.functions:
        for blk in f.blocks:
            new = []
            changed = False
            for inst in blk.instructions:
                si = inst.sync_info
                if si is not None and len(si.on_wait) > limit:
                    waits = list(si.on_wait)
                    inst.sync_info = mybir.SyncInfo(
                        on_wait=waits[:limit], on_update=list(si.on_update))
                    for w in waits[limit:]:
                        _NOP_ID[0] += 1
                        nop = mybir.InstNoOp(
                            name=f"waitnop-{_NOP_ID[0]}", ins=[], outs=[])
                        nop.engine = inst.engine
                        nop.sync_info = mybir.SyncInfo(on_wait=[w], on_update=[])
                        new.append(nop)
                    changed = True
                new.append(inst)
            if changed:
                blk.instructions = new


def build_nc():
    nc = bass.Bass()
    x_ext = nc.declare_dram_parameter("x", [T, D], F32, isOutput=False)
    w_ext = nc.declare_dram_parameter("W_qkv", [D, 3 * 512], F32, isOutput=False)
    b_ext = nc.declare_dram_parameter("b_qkv", [3 * 512], F32, isOutput=False)
    wo_ext = nc.declare_dram_parameter("W_out", [512, D], F32, isOutput=False)
    out_ext = nc.declare_dram_parameter("out", [T, D], F32, isOutput=True)

    with tile.TileContext(nc) as tc:
        with (
            tc.tile_pool(name="const", bufs=1) as constp,
            tc.tile_pool(name="big", bufs=1) as bigp,
        ):
            ident = constp.tile([128, 128], F32, tag="ident")
            make_identity(nc, ident)

            # causal 0/1 mask [128, 2x512] bf16 (head-duplicated triangle):
            # mask[p, (h, f)] = 1 if f >= p else 0 — diagonal k-tiles reduce to
            # this one pattern after live-range slicing.
            maskt = constp.tile([128, 1024], BF16, tag="maskt")
            nc.gpsimd.memset(maskt, 1.0)
            mk3 = maskt.rearrange("p (h f) -> p h f", f=512)
            nc.gpsimd.affine_select(
                out=mk3, in_=mk3,
                compare_op=AX.is_ge, fill=0.0,
                base=0, channel_multiplier=-1,
                pattern=[[0, 2], [1, 512]],
            )

            # biases: per-partition vectors for q/k feature tiles, broadcast
            # tile for V (bias along the free dv axis)
            bq_sb = constp.tile([128, NMT], F32, tag="bq")
            bk_sb = constp.tile([128, NMT], F32, tag="bk")
            nc.sync.dma_start(
                out=bq_sb, in_=b_ext[0:512].rearrange("(m p) -> p m", p=128))
            nc.sync.dma_start(
                out=bk_sb, in_=b_ext[512:1024].rearrange("(m p) -> p m", p=128))
            bv_sb = constp.tile([128, 512], F32, tag="bv")
            bv_src = b_ext[1024:1536]
            nc.sync.dma_start(
                out=bv_sb,
                in_=bass.AP(tensor=bv_src.tensor, offset=bv_src.offset,
                            ap=[[0, 128]] + list(bv_src.ap)),
            )

            # persistent activations
            qT = bigp.tile([128, NMT * T], BF16, tag="qT")
            kT = bigp.tile([128, NMT * T], BF16, tag="kT")
            vaug = bigp.tile([128, (T // 128) * HLOC * VA], BF16, tag="vaug")
            attnT = bigp.tile([128, NMT * T], BF16, tag="attnT")
            woutb = bigp.tile([128, 4 * D], BF16, tag="woutb")

            # ------- phases B+C interleaved: transpose x, project, attention
            with (
                tc.tile_pool(name="proj_sb", bufs=1) as projp,
                tc.tile_pool(name="pjpsum", bufs=2, space="PSUM") as pjpsum,
            ):
                xT = projp.tile([128, NDIN * T], BF16, tag="xT")
                wbf = projp.tile([128, NDIN * 1536], BF16, tag="wbf")

                # x load + TensorE transpose (f32) + cast to bf16 on DVE;
                # staging pools close before the attention pools open
                with (
                    tc.tile_pool(name="xstage", bufs=5) as xstage,
                    tc.tile_pool(name="wstage", bufs=3) as wstage,
                    tc.tile_pool(name="tpsum", bufs=2, space="PSUM") as tpsum,
                ):
                    for tt in range(T // 128):
                        xt = xstage.tile([128, D], F32, tag="x")
                        nc.sync.dma_start(
                            out=xt, in_=x_ext[tt * 128:(tt + 1) * 128, :])
                        for g in range(2):
                            tp = tpsum.tile([128, 512], F32, tag="tp")
                            for j in range(4):
                                dj = g * 4 + j
                                nc.tensor.transpose(
                                    tp[:, j * 128:(j + 1) * 128],
                                    xt[:, dj * 128:(dj + 1) * 128], ident)
                            xT_dst = bass.AP(
                                tensor=xT.tensor,
                                offset=xT.offset + (g * 4) * T + tt * 128,
                                ap=[list(xT.ap[0]), [T, 4], [1, 128]])
                            nc.vector.tensor_copy(
                                xT_dst,
                                tp.rearrange("p (j t) -> p j t", t=128))

                    # weight loads + casts (ScalarE is idle this early)
                    for k in range(4):
                        wot = wstage.tile([128, D], F32, tag="wo")
                        nc.sync.dma_start(out=wot, in_=wo_ext[k * 128:(k + 1) * 128, :])
                        nc.scalar.copy(woutb[:, k * D:(k + 1) * D], wot)
                    for k in range(NDIN):
                        wt = wstage.tile([128, 1536], F32, tag="wqkv")
                        nc.sync.dma_start(out=wt, in_=w_ext[k * 128:(k + 1) * 128, :])
                        nc.scalar.copy(wbf[:, k * 1536:(k + 1) * 1536], wt)

                def emit_qkproj(mt):
                    for sec, dst, bias in ((0, qT, bq_sb), (1, kT, bk_sb)):
                        for n in range(NQT):
                            ps = pjpsum.tile([128, 512], F32, tag="pj")
                            for kk in range(NDIN):
                                nc.tensor.matmul(
                                    ps,
                                    lhsT=wbf[:, kk * 1536 + sec * 512 + mt * 128:
                                             kk * 1536 + sec * 512 + (mt + 1) * 128],
                                    rhs=xT[:, kk * T + n * 512: kk * T + (n + 1) * 512],
                                    start=(kk == 0), stop=(kk == NDIN - 1),
                                )
                            nc.scalar.activation(
                                dst[:, mt * T + n * 512: mt * T + (n + 1) * 512],
                                ps, mybir.ActivationFunctionType.Identity,
                                bias=bias[:, mt:mt + 1], scale=1.0)

                def emit_vproj(tts):
                    for tt in tts:
                        ps = pjpsum.tile([128, 512], F32, tag="pj")
                        for kk in range(NDIN):
                            nc.tensor.matmul(
                                ps,
                                lhsT=xT[:, kk * T + tt * 128: kk * T + (tt + 1) * 128],
                                rhs=wbf[:, kk * 1536 + 1024: kk * 1536 + 1536],
                                start=(kk == 0), stop=(kk == NDIN - 1),
                            )
                        blk = vaug[:, tt * (HLOC * VA):(tt + 1) * (HLOC * VA)]
                        blk3 = blk.rearrange("p (h c) -> p h c", c=VA)
                        nc.vector.tensor_tensor(
                            out=blk3[:, :, 0:64],
                            in0=ps.rearrange("p (h c) -> p h c", c=64),
                            in1=bv_sb.rearrange("p (h c) -> p h c", c=64),
                            op=AX.add)
                        nc.vector.memset(blk3[:, :, 64:65], 1.0)

                with (
                    tc.tile_pool(name="scps", bufs=2, space="PSUM") as scps,
                    tc.tile_pool(name="avps", bufs=2, space="PSUM") as avps,
                    tc.tile_pool(name="ptp", bufs=22) as ptp,
                    tc.tile_pool(name="accp", bufs=3) as accp,
                    tc.tile_pool(name="recp", bufs=2) as recp,
                    tc.tile_pool(name="dscr", bufs=4, space="DRAM") as dscr,
                    tc.tile_pool(name="yo", bufs=3) as yo,
                ):
                    def emit_unit(hp, qt):
                        nkt = 4 * (qt + 1)
                        accs = [avps.tile([128, 512], F32, tag="av",
                                          name=f"av{par}")
                                for par in range(2)]
                        m3 = maskt.rearrange("p (h q) -> p h q", q=512)
                        for c0 in range(0, nkt, 8):
                            chunk = list(range(c0, min(c0 + 8, nkt)))
                            pts = {}
                            for kt in chunk:
                                # diagonal k-tile i (k0 = q0+128i): only queries
                                # f >= 128i are live; compute/exp/mask that range
                                i = kt - (nkt - 4)
                                lo_q = max(0, 128 * i)
                                ps = scps.tile([128, 1024], F32, tag="sc")
                                for par in range(2):
                                    lo, hi = par * 64, par * 64 + 64
                                    nc.tensor.matmul(
                                        ps[:, par * 512 + lo_q:(par + 1) * 512],
                                        lhsT=kT[lo:hi, hp * T + kt * 128:
                                                hp * T + (kt + 1) * 128],
                                        rhs=qT[lo:hi, hp * T + qt * 512 + lo_q:
                                               hp * T + (qt + 1) * 512],
                                        start=True, stop=True,
                                    )
                                pt_t = ptp.tile([128, 1024], BF16, tag="pt")
                                ps3 = ps.rearrange("p (h q) -> p h q", q=512)
                                pt3 = pt_t.rearrange("p (h q) -> p h q", q=512)
                                nc.scalar.activation(
                                    pt3[:, :, lo_q:512], ps3[:, :, lo_q:512],
                                    EXP, bias=0.0, scale=0.125)
                                if i >= 0:
                                    nc.vector.tensor_tensor(
                                        out=pt3[:, :, lo_q:512],
                                        in0=pt3[:, :, lo_q:512],
                                        in1=m3[:, :, 0:512 - lo_q], op=AX.mult)
                                pts[kt] = (pt_t, lo_q)
                            for par in range(2):
                                h = 2 * hp + par
                                for kt in chunk:
                                    pt_t, lo_q = pts[kt]
                                    nc.tensor.matmul(
                                        accs[par][0:VA, lo_q:512],
                                        lhsT=vaug[:, kt * (HLOC * VA) + h * VA:
                                                  kt * (HLOC * VA) + (h + 1) * VA],
                                        rhs=pt_t[:, par * 512 + lo_q:(par + 1) * 512],
                                        start=(kt == 0), stop=(kt == nkt - 1),
                                    )
                        for par in range(2):
                            acc = accs[par]
                            accsb = accp.tile([VA, 512], F32, tag="accs")
                            nc.vector.tensor_copy(accsb, acc[0:VA, :])
                            rc = recp.tile([1, 512], F32, tag="rc")
                            rb = recp.tile([64, 512], F32, tag="rb")
                            nc.vector.reciprocal(rc, accsb[64:65, :])
                            rd = dscr.tile([1, 512], F32, tag="rd")
                            nc.sync.dma_start(out=rd, in_=rc)
                            nc.sync.dma_start(
                                out=rb,
                                in_=bass.AP(tensor=rd.tensor, offset=rd.offset,
                                            ap=[[0, 64]] + list(rd.ap)[1:]))
                            nc.vector.tensor_tensor(
                                out=attnT[par * 64:(par + 1) * 64,
                                          hp * T + qt * 512: hp * T + (qt + 1) * 512],
                                in0=accsb[0:64, :], in1=rb, op=AX.mult)

                    def emit_outproj(mts):
                        for mt in mts:
                            for n in range(2):
                                ps = pjpsum.tile([128, 512], F32, tag="pj",
                                                 name="y")
                                for kk in range(4):
                                    nc.tensor.matmul(
                                        ps,
                                        lhsT=attnT[:, kk * T + mt * 128:
                                                   kk * T + (mt + 1) * 128],
                                        rhs=woutb[:, kk * D + n * 512:
                                                  kk * D + (n + 1) * 512],
                                        start=(kk == 0), stop=(kk == 3))
                                yt = yo.tile([128, 512], F32, tag="yt",
                                             name="yt")
                                nc.vector.tensor_copy(yt, ps)
                                nc.sync.dma_start(
                                    out=out_ext[mt * 128:(mt + 1) * 128,
                                                n * 512:(n + 1) * 512], in_=yt)

                    emit_qkproj(0)
                    for hp in range(HLOC // 2):
                        if hp >= 1:
                            emit_qkproj(hp)
                        for qt in range(NQT):
                            if hp == 0:
                                emit_vproj(range(4 * qt, 4 * qt + 4))
                            emit_unit(hp, qt)
                            if hp == 3 and qt >= 1:
                                emit_outproj(range(4 * (qt - 1), 4 * qt))
                    emit_outproj(range(12, 16))

    _split_multiwaits(nc)
    return nc


_NC_CACHE = {}


def get_nc():
    if "nc" not in _NC_CACHE:
        _NC_CACHE["nc"] = build_nc()
    return _NC_CACHE["nc"]


def make_in_maps(x, W_qkv, b_qkv, W_out):
    in_maps = []
    for c in range(8):
        b, g = c // 2, c % 2
        s = slice(512 * g, 512 * (g + 1))
        wslice = np.concatenate(
            [W_qkv[:, 512 * g:512 * (g + 1)],
             W_qkv[:, 1024 + 512 * g:1024 + 512 * (g + 1)],
             W_qkv[:, 2048 + 512 * g:2048 + 512 * (g + 1)]], axis=1)
        bslice = np.concatenate(
            [b_qkv[512 * g:512 * (g + 1)],
             b_qkv[1024 + 512 * g:1024 + 512 * (g + 1)],
             b_qkv[2048 + 512 * g:2048 + 512 * (g + 1)]])
        in_maps.append({
            "x": np.ascontiguousarray(x[b], dtype=np.float32),
            "W_qkv": np.ascontiguousarray(wslice, dtype=np.float32),
            "b_qkv": np.ascontiguousarray(bslice, dtype=np.float32),
            "W_out": np.ascontiguousarray(W_out[s], dtype=np.float32),
        })
    return in_maps


def kernel(x, W_qkv, b_qkv, W_out, b_out):
    x = np.asarray(x)
    W_qkv = np.asarray(W_qkv)
    b_qkv = np.asarray(b_qkv)
    W_out = np.asarray(W_out)
    b_out = np.asarray(b_out)
    nc = get_nc()
    in_maps = make_in_maps(x, W_qkv, b_qkv, W_out)
    res = run_bass_kernel_spmd(nc, in_maps, core_ids=list(range(8))).results
    out = np.stack(
        [res[2 * b]["out"] + res[2 * b + 1]["out"] for b in range(4)], axis=0)
    out = out + b_out[None, None, :]
    return out.astype(np.float32)


# revision 37
# speedup vs baseline: 1.1897x; 1.0027x over previous
"""Causal self-attention (B=4, T=2048, D=1024, H=16) on 8 TRN2 NeuronCores.

Sharding: core c -> (batch b = c//2, head-group g = c%2 of 8 heads).
Each core computes QKV projection for its 8 heads, causal attention, and a
partial out-projection (its heads' rows of W_out). The two partials per batch
are summed on the host during unshard (the "all-reduce after out_proj" of the
tensor-parallel scheme, done host-side since on-device 2-rank collectives are
slower than the host add).

Per-core program (identical SPMD on all 8 cores):
  1. Load x [2048,1024] f32, transpose on TensorE -> xT bf16 [1024(d), 2048(t)]
  2. qT/kT = (Wq|Wk)^T-stationary matmuls -> [512(feat), 2048(t)] bf16
     V     = xT-stationary matmuls -> [2048(t), 512(dv)] bf16, augmented with a
             ones column per head (softmax denominator via the same AV matmul)
  3. Per (head-pair, q-tile of 512): scoresT [k,q] in PSUM (2 heads row-packed
     into the 64x128 PE tiling via partition-half placement), exp on ScalarE
     (scale=1/8, fp32 in -> bf16 out), causal 0/1-mask multiply on diagonal
     tiles (live query sub-ranges only), AV matmuls accumulate [65, 512]
     (64 dv rows + the softmax-denominator row from the ones column),
     normalize via VectorE reciprocal + DRAM-bounce broadcast DMA + VectorE
     multiply -> attnT bf16 [dv, q].
  4. out_proj: attnT-stationary matmuls vs W_out rows -> y partial, DMA out.
     Emitted one q-tile behind the last head-pair's attention so it fills
     TensorE stalls and hides the tail.

Projections for head-pair j+1 are emitted between attention units so the
TensorE fills exp-latency stalls with projection matmuls. Peak engine usage
(cost-model): TensorE ~242us busy of ~297us total; ScalarE ~172us; VectorE
~150us; DMA ~83us.
"""

import numpy as np

import concourse.bass as bass
import concourse.mybir as mybir
import concourse.tile as tile
from concourse.bass_utils import run_bass_kernel_spmd
from concourse.masks import make_identity

F32 = mybir.dt.float32
BF16 = mybir.dt.bfloat16
AX = mybir.AluOpType

T = 2048
D = 1024
HLOC = 8          # heads per core
DKH = 64
QT = 512          # query tile
NQT = T // QT     # 4
KT = 128          # key tile
NDIN = D // 128   # 8
NMT = 4           # q/k feature m-tiles (512 local feats / 128)
VA = 65           # V cols per head incl. ones column
EXP = mybir.ActivationFunctionType.Exp


_NOP_ID = [0]


def _split_multiwaits(nc, limit=1):
    """This toolchain's walrus rejects more than one sync-wait on an
    instruction ("Too many sync wait commands"). Move excess waits onto
    same-engine NOPs inserted immediately before the instruction — the
    engine sequencer executes them in program order, so semantics are
    preserved (issue-after-wait implies execute-after-wait for DMA too)."""
    for f in nc.m.functions:
        for blk in f.blocks:
            new = []
            changed = False
            for inst in blk.instructions:
                si = inst.sync_info
                if si is not None and len(si.on_wait) > limit:
                    waits = list(si.on_wait)
                    inst.sync_info = mybir.SyncInfo(
                        on_wait=waits[:limit], on_update=list(si.on_update))
                    for w in waits[limit:]:
                        _NOP_ID[0] += 1
                        nop = mybir.InstNoOp(
                            name=f"waitnop-{_NOP_ID[0]}", ins=[], outs=[])
                        nop.engine = inst.engine
                        nop.sync_info = mybir.SyncInfo(on_wait=[w], on_update=[])
                        new.append(nop)
                    changed = True
                new.append(inst)
            if changed:
                blk.instructions = new


def build_nc():
    nc = bass.Bass()
    x_ext = nc.declare_dram_parameter("x", [T, D], F32, isOutput=False)
    w_ext = nc.declare_dram_parameter("W_qkv", [D, 3 * 512], F32, isOutput=False)
    b_ext = nc.declare_dram_parameter("b_qkv", [3 * 512], F32, isOutput=False)
    wo_ext = nc.declare_dram_parameter("W_out", [512, D], F32, isOutput=False)
    out_ext = nc.declare_dram_parameter("out", [T, D], F32, isOutput=True)

    with tile.TileContext(nc) as tc:
        with (
            tc.tile_pool(name="const", bufs=1) as constp,
            tc.tile_pool(name="big", bufs=1) as bigp,
        ):
            ident = constp.tile([128, 128], F32, tag="ident")
            make_identity(nc, ident)

            # causal 0/1 mask [128, 2x512] bf16 (head-duplicated triangle):
            # mask[p, (h, f)] = 1 if f >= p else 0 — diagonal k-tiles reduce to
            # this one pattern after live-range slicing.
            maskt = constp.tile([128, 1024], BF16, tag="maskt")
            nc.gpsimd.memset(maskt, 1.0)
            mk3 = maskt.rearrange("p (h f) -> p h f", f=512)
            nc.gpsimd.affine_select(
                out=mk3, in_=mk3,
                compare_op=AX.is_ge, fill=0.0,
                base=0, channel_multiplier=-1,
                pattern=[[0, 2], [1, 512]],
            )

            # biases: per-partition vectors for q/k feature tiles, broadcast
            # tile for V (bias along the free dv axis)
            bq_sb = constp.tile([128, NMT], F32, tag="bq")
            bk_sb = constp.tile([128, NMT], F32, tag="bk")
            nc.sync.dma_start(
                out=bq_sb, in_=b_ext[0:512].rearrange("(m p) -> p m", p=128))
            nc.sync.dma_start(
                out=bk_sb, in_=b_ext[512:1024].rearrange("(m p) -> p m", p=128))
            bv_sb = constp.tile([128, 512], F32, tag="bv")
            bv_src = b_ext[1024:1536]
            nc.sync.dma_start(
                out=bv_sb,
                in_=bass.AP(tensor=bv_src.tensor, offset=bv_src.offset,
                            ap=[[0, 128]] + list(bv_src.ap)),
            )

            # persistent activations
            qT = bigp.tile([128, NMT * T], BF16, tag="qT")
            kT = bigp.tile([128, NMT * T], BF16, tag="kT")
            vaug = bigp.tile([128, (T // 128) * HLOC * VA], BF16, tag="vaug")
            attnT = bigp.tile([128, NMT * T], BF16, tag="attnT")
            woutb = bigp.tile([128, 4 * D], BF16, tag="woutb")

            # ------- phases B+C interleaved: transpose x, project, attention
            with (
                tc.tile_pool(name="proj_sb", bufs=1) as projp,
                tc.tile_pool(name="pjpsum", bufs=2, space="PSUM") as pjpsum,
            ):
                xT = projp.tile([128, NDIN * T], BF16, tag="xT")
                wbf = projp.tile([128, NDIN * 1536], BF16, tag="wbf")

                # x load + TensorE transpose (f32) + cast to bf16 on DVE;
                # staging pools close before the attention pools open
                with (
                    tc.tile_pool(name="xstage", bufs=5) as xstage,
                    tc.tile_pool(name="wstage", bufs=3) as wstage,
                    tc.tile_pool(name="tpsum", bufs=2, space="PSUM") as tpsum,
                ):
                    for tt in range(T // 128):
                        xt = xstage.tile([128, D], F32, tag="x")
                        nc.sync.dma_start(
                            out=xt, in_=x_ext[tt * 128:(tt + 1) * 128, :])
                        for g in range(2):
                            tp = tpsum.tile([128, 512], F32, tag="tp")
                            for j in range(4):
                                dj = g * 4 + j
                                nc.tensor.transpose(
                                    tp[:, j * 128:(j + 1) * 128],
                                    xt[:, dj * 128:(dj + 1) * 128], ident)
                            xT_dst = bass.AP(
                                tensor=xT.tensor,
                                offset=xT.offset + (g * 4) * T + tt * 128,
                                ap=[list(xT.ap[0]), [T, 4], [1, 128]])
                            nc.vector.tensor_copy(
                                xT_dst,
                                tp.rearrange("p (j t) -> p j t", t=128))

                    # weight loads + casts (ScalarE is idle this early)
                    for k in range(4):
                        wot = wstage.tile([128, D], F32, tag="wo")
                        nc.sync.dma_start(out=wot, in_=wo_ext[k * 128:(k + 1) * 128, :])
                        nc.scalar.copy(woutb[:, k * D:(k + 1) * D], wot)
                    for k in range(NDIN):
                        wt = wstage.tile([128, 1536], F32, tag="wqkv")
                        nc.sync.dma_start(out=wt, in_=w_ext[k * 128:(k + 1) * 128, :])
                        nc.scalar.copy(wbf[:, k * 1536:(k + 1) * 1536], wt)

                def emit_qkproj(mt, ns=None):
                    for sec, dst, bias in ((0, qT, bq_sb), (1, kT, bk_sb)):
                        for n in (range(NQT) if ns is None else ns):
                            ps = pjpsum.tile([128, 512], F32, tag="pj")
                            for kk in range(NDIN):
                                nc.tensor.matmul(
                                    ps,
                                    lhsT=wbf[:, kk * 1536 + sec * 512 + mt * 128:
                                             kk * 1536 + sec * 512 + (mt + 1) * 128],
                                    rhs=xT[:, kk * T + n * 512: kk * T + (n + 1) * 512],
                                    start=(kk == 0), stop=(kk == NDIN - 1),
                                )
                            nc.scalar.activation(
                                dst[:, mt * T + n * 512: mt * T + (n + 1) * 512],
                                ps, mybir.ActivationFunctionType.Identity,
                                bias=bias[:, mt:mt + 1], scale=1.0)

                def emit_vproj(tts):
                    for tt in tts:
                        ps = pjpsum.tile([128, 512], F32, tag="pj")
                        for kk in range(NDIN):
                            nc.tensor.matmul(
                                ps,
                                lhsT=xT[:, kk * T + tt * 128: kk * T + (tt + 1) * 128],
                                rhs=wbf[:, kk * 1536 + 1024: kk * 1536 + 1536],
                                start=(kk == 0), stop=(kk == NDIN - 1),
                            )
                        blk = vaug[:, tt * (HLOC * VA):(tt + 1) * (HLOC * VA)]
                        blk3 = blk.rearrange("p (h c) -> p h c", c=VA)
                        nc.vector.tensor_tensor(
                            out=blk3[:, :, 0:64],
                            in0=ps.rearrange("p (h c) -> p h c", c=64),
                            in1=bv_sb.rearrange("p (h c) -> p h c", c=64),
                            op=AX.add)
                        nc.vector.memset(blk3[:, :, 64:65], 1.0)

                with (
                    tc.tile_pool(name="scps", bufs=2, space="PSUM") as scps,
                    tc.tile_pool(name="avps", bufs=2, space="PSUM") as avps,
                    tc.tile_pool(name="ptp", bufs=22) as ptp,
                    tc.tile_pool(name="accp", bufs=3) as accp,
                    tc.tile_pool(name="recp", bufs=2) as recp,
                    tc.tile_pool(name="dscr", bufs=4, space="DRAM") as dscr,
                    tc.tile_pool(name="yo", bufs=3) as yo,
                ):
                    def emit_unit(hp, qt):
                        nkt = 4 * (qt + 1)
                        accs = [avps.tile([128, 512], F32, tag="av",
                                          name=f"av{par}")
                                for par in range(2)]
                        m3 = maskt.rearrange("p (h q) -> p h q", q=512)
                        for c0 in range(0, nkt, 8):
                            chunk = list(range(c0, min(c0 + 8, nkt)))
                            pts = {}
                            for kt in chunk:
                                # diagonal k-tile i (k0 = q0+128i): only queries
                                # f >= 128i are live; compute/exp/mask that range
                                i = kt - (nkt - 4)
                                lo_q = max(0, 128 * i)
                                ps = scps.tile([128, 1024], F32, tag="sc")
                                for par in range(2):
                                    lo, hi = par * 64, par * 64 + 64
                                    nc.tensor.matmul(
                                        ps[:, par * 512 + lo_q:(par + 1) * 512],
                                        lhsT=kT[lo:hi, hp * T + kt * 128:
                                                hp * T + (kt + 1) * 128],
                                        rhs=qT[lo:hi, hp * T + qt * 512 + lo_q:
                                               hp * T + (qt + 1) * 512],
                                        start=True, stop=True,
                                    )
                                pt_t = ptp.tile([128, 1024], BF16, tag="pt")
                                ps3 = ps.rearrange("p (h q) -> p h q", q=512)
                                pt3 = pt_t.rearrange("p (h q) -> p h q", q=512)
                                nc.scalar.activation(
                                    pt3[:, :, lo_q:512], ps3[:, :, lo_q:512],
                                    EXP, bias=0.0, scale=0.125)
                                if i >= 0:
                                    nc.vector.tensor_tensor(
                                        out=pt3[:, :, lo_q:512],
                                        in0=pt3[:, :, lo_q:512],
                                        in1=m3[:, :, 0:512 - lo_q], op=AX.mult)
                                pts[kt] = (pt_t, lo_q)
                            for par in range(2):
                                h = 2 * hp + par
                                for kt in chunk:
                                    pt_t, lo_q = pts[kt]
                                    nc.tensor.matmul(
                                        accs[par][0:VA, lo_q:512],
                                        lhsT=vaug[:, kt * (HLOC * VA) + h * VA:
                                                  kt * (HLOC * VA) + (h + 1) * VA],
                                        rhs=pt_t[:, par * 512 + lo_q:(par + 1) * 512],
                                        start=(kt == 0), stop=(kt == nkt - 1),
                                    )
                        for par in range(2):
                            acc = accs[par]
                            accsb = accp.tile([VA, 512], F32, tag="accs")
                            nc.vector.tensor_copy(accsb, acc[0:VA, :])
                            rc = recp.tile([1, 512], F32, tag="rc")
                            rb = recp.tile([64, 512], F32, tag="rb")
                            nc.vector.reciprocal(rc, accsb[64:65, :])
                            rd = dscr.tile([1, 512], F32, tag="rd")
                            nc.sync.dma_start(out=rd, in_=rc)
                            nc.sync.dma_start(
                                out=rb,
                                in_=bass.AP(tensor=rd.tensor, offset=rd.offset,
                                            ap=[[0, 64]] + list(rd.ap)[1:]))
                            nc.vector.tensor_tensor(
                                out=attnT[par * 64:(par + 1) * 64,
                                          hp * T + qt * 512: hp * T + (qt + 1) * 512],
                                in0=accsb[0:64, :], in1=rb, op=AX.mult)

                    def emit_outproj(mts):
                        for mt in mts:
                            for n in range(2):
                                ps = pjpsum.tile([128, 512], F32, tag="pj",
                                                 name="y")
                                for kk in range(4):
                                    nc.tensor.matmul(
                                        ps,
                                        lhsT=attnT[:, kk * T + mt * 128:
                                                   kk * T + (mt + 1) * 128],
                                        rhs=woutb[:, kk * D + n * 512:
                                                  kk * D + (n + 1) * 512],
                                        start=(kk == 0), stop=(kk == 3))
                                yt = yo.tile([128, 512], F32, tag="yt",
                                             name="yt")
                                nc.vector.tensor_copy(yt, ps)
                                nc.sync.dma_start(
                                    out=out_ext[mt * 128:(mt + 1) * 128,
                                                n * 512:(n + 1) * 512], in_=yt)

                    for hp in range(HLOC // 2):
                        if hp >= 1:
                            emit_qkproj(hp)
                        for qt in range(NQT):
                            if hp == 0:
                                emit_qkproj(0, ns=[qt])
                                emit_vproj(range(4 * qt, 4 * qt + 4))
                            emit_unit(hp, qt)
                            if hp == 3 and qt >= 1:
                                emit_outproj(range(4 * (qt - 1), 4 * qt))
                    emit_outproj(range(12, 16))

    _split_multiwaits(nc)
    return nc


_NC_CACHE = {}


def get_nc():
    if "nc" not in _NC_CACHE:
        _NC_CACHE["nc"] = build_nc()
    return _NC_CACHE["nc"]


def make_in_maps(x, W_qkv, b_qkv, W_out):
    in_maps = []
    for c in range(8):
        b, g = c // 2, c % 2
        s = slice(512 * g, 512 * (g + 1))
        wslice = np.concatenate(
            [W_qkv[:, 512 * g:512 * (g + 1)],
             W_qkv[:, 1024 + 512 * g:1024 + 512 * (g + 1)],
             W_qkv[:, 2048 + 512 * g:2048 + 512 * (g + 1)]], axis=1)
        bslice = np.concatenate(
            [b_qkv[512 * g:512 * (g + 1)],
             b_qkv[1024 + 512 * g:1024 + 512 * (g + 1)],
             b_qkv[2048 + 512 * g:2048 + 512 * (g + 1)]])
        in_maps.append({
            "x": np.ascontiguousarray(x[b], dtype=np.float32),
            "W_qkv": np.ascontiguousarray(wslice, dtype=np.float32),
            "b_qkv": np.ascontiguousarray(bslice, dtype=np.float32),
            "W_out": np.ascontiguousarray(W_out[s], dtype=np.float32),
        })
    return in_maps


def kernel(x, W_qkv, b_qkv, W_out, b_out):
    x = np.asarray(x)
    W_qkv = np.asarray(W_qkv)
    b_qkv = np.asarray(b_qkv)
    W_out = np.asarray(W_out)
    b_out = np.asarray(b_out)
    nc = get_nc()
    in_maps = make_in_maps(x, W_qkv, b_qkv, W_out)
    res = run_bass_kernel_spmd(nc, in_maps, core_ids=list(range(8))).results
    out = np.stack(
        [res[2 * b]["out"] + res[2 * b + 1]["out"] for b in range(4)], axis=0)
    out = out + b_out[None, None, :]
    return out.astype(np.float32)
